# revision 1
# baseline (speedup 1.0000x reference)
"""IsoGMM loss kernel for 8 Trainium2 NeuronCores.

loss = mean_{n,k} r[n,k] * ||X[n] - mus[k]||^2

Decomposition (the loss folds into two accumulated PE matmuls per core):
  sum_{n,k} r*d2 = T1 + T2 - 2*T3
    T1 = sum_n xsq_n * R_n        (xsq_n = ||X[n]||^2, R_n = sum_k r[n,k])
    T2 = sum_k musq_k * C_k       (C_k = sum_n r[n,k])
    T3 = sum_{k,d} mus[k,d] * M[k,d],  M = r.T @ X

Host augments X rows to width 129: [X | 1]. Per pair of 128-row
segments, two DoubleRow matmuls (256-row contraction each):
  ps [64,129] += sum_i r_pair[:,i,:].T @ [X | 1]_pair[:,i,:]
  ps2[64,129] += sum_i r_pair[:,i,:].T @ ([X | 1]^2)_pair[:,i,:]
ps cols 0:128 = M, col 128 = C_k; sum_d ps2[k,d] = sum_n r[n,k]*
(xsq_n + 1) - the T1 ingredient, with the PE doing the per-row reduce
for free as part of the contraction. The stray +1 per row (squared
ones-column) is cancelled exactly by the musq-1 coefficient in the
host-side combine.

The elementwise square is the only remaining vector-engine pass; every
X element must cross a multiplier once and the PE can't square, so the
pass is split across DVE (~1.4ns/elem fp8), GPSIMD (~2.1ns/elem) and -
experimentally - the scalar/ACT engine per the SQ_* chunk maps.

X and r are shipped in fp8 e4m3 (the 2e-2 rel-err budget dwarfs fp8
noise, simulated+measured ~4e-3): quarters HBM traffic vs f32 and
enables the PE DoubleRow perf mode. X alternates between the sync- and
scalar-issued DMA queues; r rides the gpsimd-issued one.

Sharding: data-parallel over N, 16384 rows per core. Each SBUF
partition holds 128 *contiguous* rows (row order is irrelevant for
every term), so every DMA is perfectly contiguous per partition. Chunk
sizes taper at the ends (fast pipeline fill after the ~7us engine-init
preamble, short tail chain before the out DMA).
"""

import numpy as np
import ml_dtypes

import concourse.bass as bass
import concourse.mybir as mybir
import concourse.tile as tile
from concourse import bacc
from concourse.bass_utils import run_bass_kernel_spmd

N, K, D = 131072, 64, 128
NCORES = 8
W = D + 1            # augmented row width: 128 data + ones
NS = N // NCORES     # rows per core
RPP = NS // 128      # rows per SBUF partition (= segments per core)
# The DMA queues are packet-rate-bound (~20ns per 128-partition line
# regardless of size, saturating ~230 GB/s only at 4KB lines), so
# chunks are as big as possible: 32 segs = 4128B lines. Total X
# packets = 128 * n_chunks, split over two queues.
CHUNK_SIZES = (16, 32, 32, 32, 16)
# Every chunk's square splits three ways (DVE/ACT/GPSIMD slices, even-
# aligned) so the engines track DMA arrivals with no backlog and the
# trailing matmuls can start on a finished slice early. Shares follow
# measured rates net of each engine's descriptor-issue load:
# DVE ~41%, ACT ~36%, GPSIMD the rest.
SQ_DVE_FRAC, SQ_ACT_FRAC = 0.41, 0.36
# r ships in 2 big DMAs (4096B lines), issued by GPSIMD so the scalar
# engine keeps its cycles for ACT squares and X descriptors.
R_CHUNKS = ((0, 64), (64, 64))  # (start_seg, n_segs)

FP8 = mybir.dt.float8e4
NP_FP8 = ml_dtypes.float8_e4m3


def build_nc(rpp=RPP, chunk_sizes=CHUNK_SIZES):
    segs = rpp
    assert sum(chunk_sizes) == segs
    xf = rpp * W
    rf = rpp * K
    f32 = mybir.dt.float32
    DR = mybir.MatmulPerfMode.DoubleRow

    # Bacc (not plain Bass): its compile() splits sync waits to satisfy
    # TRN2's 1-wait-per-instruction limit, which walrus enforces.
    nc = bacc.Bacc("TRN2", target_bir_lowering=False, debug=False)
    xp = nc.dram_tensor("xp", [128, xf], FP8, kind="ExternalInput")
    rp = nc.dram_tensor("rp", [128, rf], FP8, kind="ExternalInput")
    out = nc.dram_tensor("out", [K, 2 * W], f32, kind="ExternalOutput")

    with (
        tile.TileContext(nc) as tc,
        # Every chunk gets its own resident buffer (~41KB/partition total,
        # well under the 208KB budget): the DMA queue never stalls waiting
        # for compute to release a tile.
        tc.tile_pool(name="xb", bufs=len(CHUNK_SIZES)) as xpool,
        tc.tile_pool(name="rb", bufs=len(R_CHUNKS)) as rpool,
        tc.tile_pool(name="scr", bufs=len(CHUNK_SIZES)) as spool,
        tc.tile_pool(name="one", bufs=1) as onepool,
        tc.tile_pool(name="ps", bufs=2, space="PSUM") as pspool,
    ):
        ps = pspool.tile([K, W], f32, tag="ps")
        ps2 = pspool.tile([K, W], f32, tag="ps2")

        r_tiles = []
        for rs, rn in R_CHUNKS:
            rt = rpool.tile([128, rn * K], FP8, tag=f"r{rs}")
            r_tiles.append((rs, rn, rt))

        def r_pair(s):
            for rs, rn, rt in r_tiles:
                if rs <= s < rs + rn:
                    r3 = rt.rearrange("p (s k) -> p s k", k=K)
                    return r3[:, s - rs:s - rs + 2, :]
            raise AssertionError(s)

        def r_dma(idx):
            rs, rn, rt = r_tiles[idx]
            nc.gpsimd.dma_start(out=rt, in_=rp[:, rs * K:(rs + rn) * K])

        # Queue plan (sync's queue measured 27-58 GB/s under load, the
        # scalar/gpsimd queues 130-210): sync carries only the small
        # first X chunk (arrives mid-stream regardless), the late chunks
        # ride the fast queues so the tail chain isn't arrival-gated.
        # gpsimd FIFO: r0, X2, r1 - r1 (segs 64+) is first used by
        # chunk 2's later pairs, just after its own DMA lands.
        r_dma(0)
        x_queues = (nc.sync, nc.scalar, nc.gpsimd, nc.scalar, nc.scalar)
        s0 = 0
        for c, spc in enumerate(chunk_sizes):
            xt = xpool.tile([128, spc * W], FP8, tag="x")
            x_queues[c].dma_start(out=xt, in_=xp[:, s0 * W:(s0 + spc) * W])
            if c == 2:
                r_dma(1)

            x3 = xt.rearrange("p (s w) -> p s w", w=W)

            sq = spool.tile([128, spc * W], FP8, tag="sq")
            sq3 = sq.rearrange("p (s w) -> p s w", w=W)
            nd = 2 * round(SQ_DVE_FRAC * spc / 2)
            na = 2 * round(SQ_ACT_FRAC * spc / 2)
            for a0, a1, eng in (
                (0, nd, nc.vector),
                (nd, nd + na, nc.scalar),
                (nd + na, spc, nc.gpsimd),
            ):
                if a1 <= a0:
                    continue
                if eng is nc.scalar:
                    eng.square(sq[:, a0 * W:a1 * W], xt[:, a0 * W:a1 * W])
                else:
                    eng.tensor_mul(
                        sq[:, a0 * W:a1 * W],
                        xt[:, a0 * W:a1 * W],
                        xt[:, a0 * W:a1 * W],
                    )

            for j in range(0, spc, 2):
                s = s0 + j
                lhsT = r_pair(s)
                nc.tensor.matmul(
                    ps,
                    lhsT=lhsT,
                    rhs=x3[:, j:j + 2, :],
                    start=(s == 0),
                    stop=(s == segs - 2),
                    perf_mode=DR,
                )
                nc.tensor.matmul(
                    ps2,
                    lhsT=lhsT,
                    rhs=sq3[:, j:j + 2, :],
                    start=(s == 0),
                    stop=(s == segs - 2),
                    perf_mode=DR,
                )
            s0 += spc

        # Ship both accumulated [K, W] panels; the final weighted sum is
        # part of host-side unsharding.
        osb = onepool.tile([K, 2 * W], f32)
        nc.vector.tensor_copy(osb[:, 0:W], ps)
        nc.vector.tensor_copy(osb[:, W:2 * W], ps2)
        nc.sync.dma_start(out=out[:, :], in_=osb)

    nc.compile()
    return nc


def make_in_maps(X, r, mus, ncores=NCORES):
    X = np.ascontiguousarray(np.asarray(X, dtype=np.float32))
    r = np.ascontiguousarray(np.asarray(r, dtype=np.float32))
    n = X.shape[0]
    ns = n // ncores

    Xb = X.astype(NP_FP8)
    rb = r.astype(NP_FP8)

    in_maps = []
    for i in range(ncores):
        Xa = np.empty((ns, W), NP_FP8)
        Xa[:, :D] = Xb[i * ns:(i + 1) * ns]
        Xa[:, D] = 1.0
        in_maps.append(
            {
                "xp": np.ascontiguousarray(Xa.reshape(128, (ns // 128) * W)),
                "rp": np.ascontiguousarray(
                    rb[i * ns:(i + 1) * ns].reshape(128, (ns // 128) * K)
                ),
            }
        )
    return in_maps


def combine_outputs(results, mus):
    """Unshard: weighted sum of each core's panels -> mean."""
    mus = np.asarray(mus, dtype=np.float32)
    musq = (mus.astype(np.float64) ** 2).sum(1)
    # col 128 coefficient is musq-1: every row's T1 contribution (the
    # ps2 row-sum) carries a stray +1 from the squared ones-column,
    # cancelled exactly by the -1 on C_k here.
    ma = np.concatenate(
        [-2.0 * mus.astype(np.float64), musq[:, None] - 1.0], axis=1
    )
    total = 0.0
    for res in results:
        panel = res["out"].astype(np.float64)
        total += float((ma * panel[:, :W]).sum()) + float(panel[:, W:].sum())
    return np.array(total / (N * K), dtype=np.float32)


def kernel(X, r, mus):
    nc = build_nc()
    in_maps = make_in_maps(X, r, mus)
    res = run_bass_kernel_spmd(nc, in_maps, list(range(NCORES)))
    return combine_outputs(res.results[:NCORES], mus)



# revision 2
# speedup vs baseline: 1.0492x; 1.0492x over previous
"""IsoGMM loss kernel for 8 Trainium2 NeuronCores.

loss = mean_{n,k} r[n,k] * ||X[n] - mus[k]||^2

Decomposition (two accumulated PE matmuls per core over 64 DoubleRow
row-pair segments; fp8 e4m3 inputs, ~4e-3 rel err vs the 2e-2 budget):
  ps  [64,129] += r_pair.T @ [X|1]_pair        (M = r.T X, col 128 = C_k)
  ps2 [64,129] += r_pair.T @ ([X|1]^2)_pair    (row-sum -> T1 + C_k)
Host combine contracts ps with mus (O(K*D)) and reads the on-chip
row-reduction of ps2.

Pipeline structure (from perfetto/ntff trace analysis):
 - Host interleaves X and r into per-block contiguous pieces
   ([X(n x 129) | r(n x 64)] per partition), so one DMA delivers
   everything a run of pairs needs - no X/r queue competition, bigger
   per-partition lines.
 - Blocks are consumed in expected ARRIVAL order, not segment order
   (the loss is a sum; PSUM accumulation order is irrelevant). The sync
   queue prefetches two late blocks early; all gpsimd square work rides
   them (gpsimd needs ~2.8us lead per 12-seg slice).
 - DMA sems fire on the slowest of 16 engine rows, so under cross-queue
   contention the early window uses small 8-seg pieces round-robined
   across the scalar/gpsimd queues, growing to 12-16 segs later.
 - Per block the ps matmuls (gated only on DMA) issue immediately; the
   ps2 matmuls (gated on squares) are deferred one block, and dummy
   filler matmuls pad early DMA gaps: an idle PE drops its p-state
   (57ns -> 108ns per matmul), so the PE is never allowed to idle.
 - Squares split across ACT/DVE/GPSIMD by measured rates (~138/165/270
   ns per 129-elem segment with amortized per-instr overhead).
 - Output [64,130]: ps panel (shipped as soon as ps closes, overlapping
   the ps2 tail) + on-chip row-sums of ps2.
"""

import numpy as np
import ml_dtypes

import concourse.bass as bass
import concourse.mybir as mybir
import concourse.tile as tile
from concourse import bacc
from concourse.bass_utils import run_bass_kernel_spmd

N, K, D = 131072, 64, 128
NCORES = 8
W = D + 1
NS = N // NCORES
SEGS = NS // 128          # 128 segments of 128 rows
BW = W + K                # combined bytes per seg per partition (193)

FP8 = mybir.dt.float8e4
NP_FP8 = ml_dtypes.float8_e4m3

# blocks: (start_seg, n_segs, queue, square slices) in CONSUMPTION order.
# queue: 'sp' (sync), 'act' (scalar), 'gps' (gpsimd).
# Tapered sizes: small head (fast first pair), 16-seg steady state (sem
# granularity tracks the stream), small tail (tiny post-DMA square tail).
# The two sp blocks are consumed near the end but prefetched first on the
# idle sync queue, so ALL gpsimd square work (slow: ~335ns/seg + 500ns
# launch) rides them with ~6us of lead time.
# Listed in CONSUMPTION order (expected readiness order, not seg order).
# FILLERS: dummy matmuls inserted after this block's ps2 group to keep
# the PE busy through the early DMA ramp (an idle PE resets its p-state
# ramp back to 108ns/MM).
# DMA sems fire on the SLOWEST of 16 engine rows; under cross-queue
# contention big early pieces complete several us after most of their
# data lands. So the early window uses small 8-seg pieces round-robined
# across act/gps, growing to 12-16 segs later. sp prefetches the two
# late blocks; all gpsimd square work rides the first one.
BLOCKS = (
    #  seg   n   queue  slices                          fillers
    (0,   4,  "act", (("a", 2), ("d", 2)), 8),
    (4,   8,  "gps", (("a", 4), ("d", 4)), 8),
    (12,  8,  "act", (("a", 4), ("d", 4)), 6),
    (20,  8,  "gps", (("a", 4), ("d", 4)), 4),
    (28,  8,  "act", (("a", 4), ("d", 4)), 4),
    (36,  8,  "gps", (("a", 4), ("d", 4)), 4),
    (44,  12, "act", (("a", 6), ("d", 6)), 4),
    (56,  12, "gps", (("a", 8), ("d", 4)), 0),
    (68,  12, "act", (("a", 6), ("d", 6)), 0),
    (80,  16, "gps", (("a", 10), ("d", 6)), 0),
    (96,  16, "sp",  (("g", 16),), 0),
    (112, 16, "sp",  (("g", 8), ("a", 4), ("d", 4)), 0),
)
# per-queue DMA issue order (indices into BLOCKS), per-queue FIFO:
# sp: the gpsimd-squared block first (gpsimd is start-gated on it).
ISSUE_ORDER = (10, 0, 1, 2, 3, 11, 4, 5, 6, 7, 8, 9)

N_PE_WARM = 36
N_DVE_WARM = 0   # scheduler floats these mid-stream: they add load, not warmup
N_GPS_WARM = 1   # pulls the Q7 library load off the critical path


def _block_offsets():
    offs = {}
    o = 0
    for bs, bn, q, sl, fl in BLOCKS:
        offs[bs] = o
        o += bn * BW
    assert o == SEGS * BW
    return offs


def build_nc():
    f32 = mybir.dt.float32
    DR = mybir.MatmulPerfMode.DoubleRow

    nc = bacc.Bacc("TRN2", target_bir_lowering=False, debug=False)
    cp = nc.dram_tensor("cp", [128, SEGS * BW], FP8, kind="ExternalInput")
    out = nc.dram_tensor("out", [K, W + 1], f32, kind="ExternalOutput")

    offs = _block_offsets()

    with (
        tile.TileContext(nc) as tc,
        tc.tile_pool(name="cb", bufs=1) as cpool,
        tc.tile_pool(name="scr", bufs=1) as spool,
        tc.tile_pool(name="warm", bufs=3) as wpool,
        tc.tile_pool(name="one", bufs=1) as onepool,
        tc.tile_pool(name="ps", bufs=1, space="PSUM") as pspool,
    ):
        qmap = {"sp": nc.sync, "act": nc.scalar, "gps": nc.gpsimd}
        engmap = {"a": nc.scalar, "d": nc.vector, "g": nc.gpsimd}

        ps = pspool.tile([K, W], f32, tag="ps")
        ps2 = pspool.tile([K, W], f32, tag="ps2")
        wps = pspool.tile([K, W], f32, tag="wps")

        cpt = cpool.tile([128, SEGS * BW], FP8, tag="cp")
        sqt = spool.tile([128, SEGS * W], FP8, tag="sq")

        # warm tiles (memset, no DMA deps)
        wx = wpool.tile([128, 2 * W], FP8, tag="wx")
        wr = wpool.tile([128, 2 * K], FP8, tag="wr")
        wsq = wpool.tile([128, 2 * W], FP8, tag="wsq")
        nc.gpsimd.memset(wx, 0)
        nc.gpsimd.memset(wr, 0)

        # ---- all input DMAs up front ----
        for bi in ISSUE_ORDER:
            bs, bn, q, _, _ = BLOCKS[bi]
            o = offs[bs]
            qmap[q].dma_start(
                out=cpt[:, o:o + bn * BW], in_=cp[:, o:o + bn * BW]
            )

        # ---- engine warmups (pre-data) ----
        wx3 = wx.rearrange("p (s w) -> p s w", w=W)
        wr3 = wr.rearrange("p (s k) -> p s k", k=K)
        for _ in range(N_GPS_WARM):
            nc.gpsimd.tensor_mul(wsq[:, 0:W], wx[:, 0:W], wx[:, 0:W])
        for _ in range(N_DVE_WARM):
            nc.vector.tensor_mul(wsq[:, 0:W], wx[:, 0:W], wx[:, 0:W])
        for _ in range(N_PE_WARM):
            nc.tensor.matmul(
                wps, lhsT=wr3, rhs=wx3, start=True, stop=True, perf_mode=DR
            )

        sq3 = sqt.rearrange("p (s w) -> p s w", w=W)

        # ---- main pipeline, blocks in consumption (arrival) order ----
        # Per block: emit the ps matmuls (gated only on the block's DMA)
        # immediately, and DEFER the ps2 matmuls (gated on squares) by one
        # block. The PE then always has DMA-gated work while squares cook,
        # so it never stalls and its p-state ramp (108ns -> 57ns per MM)
        # is not reset mid-stream.
        def views(bs, bn):
            o = offs[bs]
            xb3 = cpt[:, o:o + bn * W].rearrange("p (s w) -> p s w", w=W)
            rb3 = cpt[:, o + bn * W:o + bn * BW].rearrange(
                "p (s k) -> p s k", k=K
            )
            return xb3, rb3

        nblocks = len(BLOCKS)

        def ps_mms(bi, start):
            bs, bn, _, _, _ = BLOCKS[bi]
            xb3, rb3 = views(bs, bn)
            for j in range(0, bn, 2):
                nc.tensor.matmul(
                    ps,
                    lhsT=rb3[:, j:j + 2, :],
                    rhs=xb3[:, j:j + 2, :],
                    start=start and j == 0,
                    stop=(bi == nblocks - 1) and j == bn - 2,
                    perf_mode=DR,
                )

        def ps2_mms(bi, start):
            bs, bn, _, _, _ = BLOCKS[bi]
            _, rb3 = views(bs, bn)
            for j in range(0, bn, 2):
                nc.tensor.matmul(
                    ps2,
                    lhsT=rb3[:, j:j + 2, :],
                    rhs=sq3[:, bs + j:bs + j + 2, :],
                    start=start and j == 0,
                    stop=(bi == nblocks - 1) and j == bn - 2,
                    perf_mode=DR,
                )

        for bi, (bs, bn, q, slices, fillers) in enumerate(BLOCKS):
            assert sum(n for _, n in slices) == bn
            o = offs[bs]
            a0 = 0
            for ename, n in slices:
                a1 = a0 + n
                eng = engmap[ename]
                src = cpt[:, o + a0 * W:o + a1 * W]
                dst = sqt[:, (bs + a0) * W:(bs + a1) * W]
                if eng is nc.scalar:
                    eng.square(dst, src)
                else:
                    eng.tensor_mul(dst, src, src)
                a0 = a1

            ps_mms(bi, start=(bi == 0))
            if bi > 0:
                ps2_mms(bi - 1, start=(bi == 1))
            for _ in range(fillers):
                nc.tensor.matmul(
                    wps, lhsT=wr3, rhs=wx3, start=True, stop=True,
                    perf_mode=DR,
                )
        ps2_mms(nblocks - 1, start=False)

        # ---- evac + out: ps panel + row-sums of ps2 ----
        osb = onepool.tile([K, W + 1], f32)
        nc.vector.tensor_copy(osb[:, 0:W], ps)
        nc.vector.tensor_reduce(
            osb[:, W:W + 1], ps2, axis=mybir.AxisListType.X,
            op=mybir.AluOpType.add,
        )
        nc.scalar.dma_start(out=out[:, :], in_=osb)

    nc.compile()
    return nc


def make_in_maps(X, r, mus, ncores=NCORES):
    X = np.ascontiguousarray(np.asarray(X, dtype=np.float32))
    r = np.ascontiguousarray(np.asarray(r, dtype=np.float32))
    n = X.shape[0]
    ns = n // ncores

    Xb = X.astype(NP_FP8)
    rb = r.astype(NP_FP8)

    in_maps = []
    for i in range(ncores):
        Xa = np.empty((128, ns // 128, W), NP_FP8)
        Xa[:, :, :D] = Xb[i * ns:(i + 1) * ns].reshape(128, ns // 128, D)
        Xa[:, :, D] = 1.0
        rc = rb[i * ns:(i + 1) * ns].reshape(128, ns // 128, K)

        cpb = np.empty((128, (ns // 128) * BW), NP_FP8)
        o = 0
        for bs, bn, q, sl, fl in BLOCKS:
            cpb[:, o:o + bn * W] = Xa[:, bs:bs + bn].reshape(128, bn * W)
            o += bn * W
            cpb[:, o:o + bn * K] = rc[:, bs:bs + bn].reshape(128, bn * K)
            o += bn * K
        in_maps.append({"cp": np.ascontiguousarray(cpb)})
    return in_maps


def combine_outputs(results, mus):
    mus = np.asarray(mus, dtype=np.float32)
    musq = (mus.astype(np.float64) ** 2).sum(1)
    # out panel: cols 0:W = ps (r.T @ [X|1]); col W = row-sums of ps2,
    # i.e. sum_n r[n,k]*(xsq_n + 1) per k (the +1 from the squared ones
    # column is cancelled by the musq-1 coefficient on ps col 128).
    ma = np.concatenate(
        [-2.0 * mus.astype(np.float64), musq[:, None] - 1.0], axis=1
    )
    total = 0.0
    for res in results:
        panel = res["out"].astype(np.float64)
        total += float((ma * panel[:, :W]).sum()) + float(panel[:, W].sum())
    return np.array(total / (N * K), dtype=np.float32)


def kernel(X, r, mus):
    nc = build_nc()
    in_maps = make_in_maps(X, r, mus)
    res = run_bass_kernel_spmd(nc, in_maps, list(range(NCORES)))
    return combine_outputs(res.results[:NCORES], mus)


# revision 3
# speedup vs baseline: 1.1494x; 1.0954x over previous
"""IsoGMM loss kernel for 8 Trainium2 NeuronCores.

loss = mean_{n,k} r[n,k] * ||X[n] - mus[k]||^2

Decomposition (two accumulated PE matmuls per core over 64 DoubleRow
row-pair segments; fp8 e4m3 inputs, ~4e-3 rel err vs the 2e-2 budget):
  ps  [64,129] += r_pair.T @ [X|1]_pair        (M = r.T X, col 128 = C_k)
  ps2 [64,129] += r_pair.T @ ([X|1]^2)_pair    (row-sum -> T1 + C_k)
Host combine contracts ps with mus (O(K*D)) and reads the on-chip
row-reduction of ps2.

Pipeline structure (from perfetto/ntff trace analysis):
 - Host interleaves X and r into per-block contiguous pieces
   ([X(n x 129) | r(n x 64)] per partition), so one DMA delivers
   everything a run of pairs needs - no X/r queue competition, bigger
   per-partition lines.
 - Blocks are consumed in expected ARRIVAL order, not segment order
   (the loss is a sum; PSUM accumulation order is irrelevant). The sync
   queue prefetches two late blocks early; all gpsimd square work rides
   them (gpsimd needs ~2.8us lead per 12-seg slice).
 - DMA sems fire on the slowest of 16 engine rows, so under cross-queue
   contention the early window uses small 8-seg pieces round-robined
   across the scalar/gpsimd queues, growing to 12-16 segs later.
 - Per block the ps matmuls (gated only on DMA) issue immediately; the
   ps2 matmuls (gated on squares) are deferred one block, and dummy
   filler matmuls pad early DMA gaps: an idle PE drops its p-state
   (57ns -> 108ns per matmul), so the PE is never allowed to idle.
 - Squares split across ACT/DVE/GPSIMD by measured rates (~138/165/270
   ns per 129-elem segment with amortized per-instr overhead).
 - Output [64,130]: ps panel (shipped as soon as ps closes, overlapping
   the ps2 tail) + on-chip row-sums of ps2.
"""

import numpy as np
import ml_dtypes

import concourse.bass as bass
import concourse.mybir as mybir
import concourse.tile as tile
from concourse import bacc
from concourse.bass_utils import run_bass_kernel_spmd

N, K, D = 131072, 64, 128
NCORES = 8
W = D + 1
NS = N // NCORES
SEGS = NS // 128          # 128 segments of 128 rows
BW = W + K                # combined bytes per seg per partition (193)

FP8 = mybir.dt.float8e4
NP_FP8 = ml_dtypes.float8_e4m3

# blocks: (start_seg, n_segs, queue, square slices) in CONSUMPTION order.
# queue: 'sp' (sync), 'act' (scalar), 'gps' (gpsimd).
# Tapered sizes: small head (fast first pair), 16-seg steady state (sem
# granularity tracks the stream), small tail (tiny post-DMA square tail).
# The two sp blocks are consumed near the end but prefetched first on the
# idle sync queue, so ALL gpsimd square work (slow: ~335ns/seg + 500ns
# launch) rides them with ~6us of lead time.
# Listed in CONSUMPTION order (expected readiness order, not seg order).
# FILLERS: dummy matmuls inserted after this block's ps2 group to keep
# the PE busy through the early DMA ramp (an idle PE resets its p-state
# ramp back to 108ns/MM).
# DMA sems fire on the SLOWEST of 16 engine rows; under cross-queue
# contention big early pieces complete several us after most of their
# data lands. So the early window uses small 8-seg pieces round-robined
# across act/gps, growing to 12-16 segs later. sp prefetches the two
# late blocks; all gpsimd square work rides the first one.
BLOCKS = (
    #  seg   n   queue  slices                          fillers
    (0,   4,  "act", (("a", 2), ("d", 2)), 8),
    (4,   8,  "gps", (("a", 4), ("d", 4)), 12),
    (12,  8,  "act", (("a", 4), ("d", 4)), 6),
    (20,  8,  "gps", (("a", 4), ("d", 4)), 4),
    (28,  8,  "act", (("a", 4), ("d", 4)), 4),
    (36,  8,  "gps", (("a", 4), ("d", 4)), 8),
    (44,  12, "act", (("a", 6), ("d", 6)), 4),
    (56,  12, "gps", (("a", 8), ("d", 4)), 0),
    (68,  12, "act", (("a", 6), ("d", 6)), 0),
    (80,  16, "gps", (("a", 10), ("d", 6)), 0),
    (96,  16, "sp",  (("g", 16),), 0),
    (112, 16, "sp",  (("g", 8), ("a", 4), ("d", 4)), 0),
)
# per-queue DMA issue order (indices into BLOCKS), per-queue FIFO:
# sp: the gpsimd-squared block first (gpsimd is start-gated on it).
ISSUE_ORDER = (10, 0, 1, 2, 3, 11, 4, 5, 6, 7, 8, 9)

N_PE_WARM = 36
N_DVE_WARM = 0   # scheduler floats these mid-stream: they add load, not warmup
N_GPS_WARM = 1   # pulls the Q7 library load off the critical path


def _block_offsets():
    offs = {}
    o = 0
    for bs, bn, q, sl, fl in BLOCKS:
        offs[bs] = o
        o += bn * BW
    assert o == SEGS * BW
    return offs


def build_nc():
    f32 = mybir.dt.float32
    DR = mybir.MatmulPerfMode.DoubleRow

    nc = bacc.Bacc("TRN2", target_bir_lowering=False, debug=False)
    cp = nc.dram_tensor("cp", [128, SEGS * BW], FP8, kind="ExternalInput")
    out = nc.dram_tensor("out", [K, W + 1], f32, kind="ExternalOutput")

    offs = _block_offsets()

    with (
        tile.TileContext(nc) as tc,
        tc.tile_pool(name="cb", bufs=1) as cpool,
        tc.tile_pool(name="scr", bufs=1) as spool,
        tc.tile_pool(name="warm", bufs=3) as wpool,
        tc.tile_pool(name="one", bufs=1) as onepool,
        tc.tile_pool(name="ps", bufs=1, space="PSUM") as pspool,
    ):
        qmap = {"sp": nc.sync, "act": nc.scalar, "gps": nc.gpsimd}
        engmap = {"a": nc.scalar, "d": nc.vector, "g": nc.gpsimd}

        ps = pspool.tile([K, W], f32, tag="ps")
        ps2 = pspool.tile([K, W], f32, tag="ps2")
        wps = pspool.tile([K, W], f32, tag="wps")

        cpt = cpool.tile([128, SEGS * BW], FP8, tag="cp")
        sqt = spool.tile([128, SEGS * W], FP8, tag="sq")

        # warm tiles (memset, no DMA deps)
        wx = wpool.tile([128, 2 * W], FP8, tag="wx")
        wr = wpool.tile([128, 2 * K], FP8, tag="wr")
        wsq = wpool.tile([128, 2 * W], FP8, tag="wsq")
        nc.gpsimd.memset(wx, 0)
        nc.gpsimd.memset(wr, 0)

        # ---- all input DMAs up front ----
        for bi in ISSUE_ORDER:
            bs, bn, q, _, _ = BLOCKS[bi]
            o = offs[bs]
            qmap[q].dma_start(
                out=cpt[:, o:o + bn * BW], in_=cp[:, o:o + bn * BW]
            )

        # ---- engine warmups (pre-data) ----
        wx3 = wx.rearrange("p (s w) -> p s w", w=W)
        wr3 = wr.rearrange("p (s k) -> p s k", k=K)
        for _ in range(N_GPS_WARM):
            nc.gpsimd.tensor_mul(wsq[:, 0:W], wx[:, 0:W], wx[:, 0:W])
        for _ in range(N_DVE_WARM):
            nc.vector.tensor_mul(wsq[:, 0:W], wx[:, 0:W], wx[:, 0:W])
        for _ in range(N_PE_WARM):
            nc.tensor.matmul(
                wps, lhsT=wr3, rhs=wx3, start=True, stop=True, perf_mode=DR
            )

        sq3 = sqt.rearrange("p (s w) -> p s w", w=W)

        # ---- main pipeline, blocks in consumption (arrival) order ----
        # Per block: emit the ps matmuls (gated only on the block's DMA)
        # immediately, and DEFER the ps2 matmuls (gated on squares) by one
        # block. The PE then always has DMA-gated work while squares cook,
        # so it never stalls and its p-state ramp (108ns -> 57ns per MM)
        # is not reset mid-stream.
        def views(bs, bn):
            o = offs[bs]
            xb3 = cpt[:, o:o + bn * W].rearrange("p (s w) -> p s w", w=W)
            rb3 = cpt[:, o + bn * W:o + bn * BW].rearrange(
                "p (s k) -> p s k", k=K
            )
            return xb3, rb3

        nblocks = len(BLOCKS)

        def ps_mms(bi, start):
            bs, bn, _, _, _ = BLOCKS[bi]
            xb3, rb3 = views(bs, bn)
            for j in range(0, bn, 2):
                nc.tensor.matmul(
                    ps,
                    lhsT=rb3[:, j:j + 2, :],
                    rhs=xb3[:, j:j + 2, :],
                    start=start and j == 0,
                    stop=(bi == nblocks - 1) and j == bn - 2,
                    perf_mode=DR,
                )

        def ps2_mms(bi, start):
            bs, bn, _, _, _ = BLOCKS[bi]
            _, rb3 = views(bs, bn)
            for j in range(0, bn, 2):
                nc.tensor.matmul(
                    ps2,
                    lhsT=rb3[:, j:j + 2, :],
                    rhs=sq3[:, bs + j:bs + j + 2, :],
                    start=start and j == 0,
                    stop=(bi == nblocks - 1) and j == bn - 2,
                    perf_mode=DR,
                )

        for bi, (bs, bn, q, slices, fillers) in enumerate(BLOCKS):
            assert sum(n for _, n in slices) == bn
            o = offs[bs]
            a0 = 0
            for ename, n in slices:
                a1 = a0 + n
                eng = engmap[ename]
                src = cpt[:, o + a0 * W:o + a1 * W]
                dst = sqt[:, (bs + a0) * W:(bs + a1) * W]
                if eng is nc.scalar:
                    eng.square(dst, src)
                else:
                    eng.tensor_mul(dst, src, src)
                a0 = a1

            ps_mms(bi, start=(bi == 0))
            if bi > 0:
                ps2_mms(bi - 1, start=(bi == 1))
            for _ in range(fillers):
                nc.tensor.matmul(
                    wps, lhsT=wr3, rhs=wx3, start=True, stop=True,
                    perf_mode=DR,
                )
        ps2_mms(nblocks - 1, start=False)

        # ---- evac + out: ps panel + row-sums of ps2 ----
        osb = onepool.tile([K, W + 1], f32)
        nc.vector.tensor_copy(osb[:, 0:W], ps)
        nc.vector.tensor_reduce(
            osb[:, W:W + 1], ps2, axis=mybir.AxisListType.X,
            op=mybir.AluOpType.add,
        )
        nc.scalar.dma_start(out=out[:, :], in_=osb)

    nc.compile()
    return nc


def make_in_maps(X, r, mus, ncores=NCORES):
    X = np.ascontiguousarray(np.asarray(X, dtype=np.float32))
    r = np.ascontiguousarray(np.asarray(r, dtype=np.float32))
    n = X.shape[0]
    ns = n // ncores

    Xb = X.astype(NP_FP8)
    rb = r.astype(NP_FP8)

    in_maps = []
    for i in range(ncores):
        Xa = np.empty((128, ns // 128, W), NP_FP8)
        Xa[:, :, :D] = Xb[i * ns:(i + 1) * ns].reshape(128, ns // 128, D)
        Xa[:, :, D] = 1.0
        rc = rb[i * ns:(i + 1) * ns].reshape(128, ns // 128, K)

        cpb = np.empty((128, (ns // 128) * BW), NP_FP8)
        o = 0
        for bs, bn, q, sl, fl in BLOCKS:
            cpb[:, o:o + bn * W] = Xa[:, bs:bs + bn].reshape(128, bn * W)
            o += bn * W
            cpb[:, o:o + bn * K] = rc[:, bs:bs + bn].reshape(128, bn * K)
            o += bn * K
        in_maps.append({"cp": np.ascontiguousarray(cpb)})
    return in_maps


def combine_outputs(results, mus):
    mus = np.asarray(mus, dtype=np.float32)
    musq = (mus.astype(np.float64) ** 2).sum(1)
    # out panel: cols 0:W = ps (r.T @ [X|1]); col W = row-sums of ps2,
    # i.e. sum_n r[n,k]*(xsq_n + 1) per k (the +1 from the squared ones
    # column is cancelled by the musq-1 coefficient on ps col 128).
    ma = np.concatenate(
        [-2.0 * mus.astype(np.float64), musq[:, None] - 1.0], axis=1
    )
    total = 0.0
    for res in results:
        panel = res["out"].astype(np.float64)
        total += float((ma * panel[:, :W]).sum()) + float(panel[:, W].sum())
    return np.array(total / (N * K), dtype=np.float32)


def kernel(X, r, mus):
    nc = build_nc()
    in_maps = make_in_maps(X, r, mus)
    res = run_bass_kernel_spmd(nc, in_maps, list(range(NCORES)))
    return combine_outputs(res.results[:NCORES], mus)
